# revision 38
# baseline (speedup 1.0000x reference)
"""Trainium2 Bass kernel for nn_AttentionHead (cross-attention head).

Reference computation:
  q = input2 @ Wq + bq ; k = input1 @ Wk + bk ; v = input1 @ Wv + bv
  out = softmax(q k^T / sqrt(64)) v          # [B, S, 64]

Sharding over 8 NeuronCores: core c handles batch b = c//2, pair-rank
r = c%2; it computes the output rows for its half of the queries. Both
cores of a pair load the full (pre-transposed, bf16) input1 of their
batch and project all of K/V locally — no collectives.

Schedule shape (the end is supply-limited by the 12 MB / core HBM
stream, so the unit order is stage-outer to leave only ~1/4 of the
attention work after the last x1 byte lands):
  phase P0: all 8 kv stages x query-chunks {0,1,2} (48 units, paced by
    the x1 DMA); phase P1: query-chunk 3 (16 units, pure engine rate).
  P0's 12 AV accumulators live in ONE 2-bank PSUM tile laid out as
  [2 banks][7 slots][65] so no matmul output straddles a bank.

Per-core dataflow (matmuls bf16):
  - Q^T projection with [Wq|Wq] stationary: QT lands duplicated in both
    partition halves (moving operand of both row-packed score matmuls).
    K/V: chunk h=0 uses [Wk|Wv] (K rows 0:63, V rows 64:127), h=1 uses
    [Wv|Wk], so K^T of half h lands on partition rows h*64. PSUM is
    evacuated (bias fused) into combined ckv tiles; V^T chunks are
    PE-transposed back to k-major with a ones column appended. All
    evac/copy work alternates ScalarE/VectorE to balance the engines.
  - scores^T = KT_block.T @ QT: block pairs (h=0, h=1) are row-packed —
    two concurrent 64-contraction matmuls in disjoint PE row groups.
  - exp alternates whole groups between ScalarE (true exp) and VectorE
    (Schraudolph 2^x bit trick: Wq is pre-scaled by 128*log2e/sqrt(dk)
    on the host so the device op is a single tensor_scalar add with the
    result bits reinterpreted as bf16; the systematic multiplicative
    bias cancels in the softmax ratio). 9:7 matches the engine rates.
  - attn@V runs operand-swapped: the probs block pt[:, h, jq*128:...]
    ([128 k, 128 q], full 128-col stationary -> FWL weight loads) is
    the stationary and V|ones [128 k, 65] is the moving operand, so
    each matmul streams only 65 columns at 100% PE utilization. One
    start=True per PSUM *bank* (start clears has_written bank-wide;
    flags=0 writes overwrite-and-set where the bit is clear).
  - software pipeline: scores of unit i+LAG are emitted before AV of
    unit i so the in-order PE queue hides the exp latency; st pool has
    3 bufs so the exp engines stay saturated.
  - host does the final division by the accumulated denominator.
"""

import contextlib
import ctypes
import sys
import types

import numpy as np

import concourse.bass as bass
import concourse.tile as tile
from concourse import bacc, mybir
from concourse.bass_utils import run_bass_kernel_spmd

# ----------------------------------------------------------------------------
B_FULL = 4
S_FULL = 4096
EMB = 1024
DK = 64
N_CORES = 8

F32 = mybir.dt.float32
BF16 = mybir.dt.bfloat16
I16 = mybir.dt.int16
AF = mybir.ActivationFunctionType
ALU = mybir.AluOpType

LOG2E = 1.4426950408889634


def install_ntff_hook():
    """Provide antenv.axon_hooks with a ctypes NTFF profile hook so
    run_bass_kernel_spmd(trace=True) can report exec_time_ns."""
    if "antenv.axon_hooks" in sys.modules:
        return
    try:
        lib = ctypes.CDLL("/opt/axon/libaxon_pjrt.so")
    except OSError:
        return
    if not hasattr(lib, "axon_start_nrt_profile"):
        return
    lib.axon_start_nrt_profile.argtypes = [ctypes.POINTER(ctypes.c_int64), ctypes.c_size_t]
    lib.axon_start_nrt_profile.restype = ctypes.c_int64
    lib.axon_stop_nrt_profile.argtypes = [ctypes.c_char_p]
    lib.axon_stop_nrt_profile.restype = ctypes.c_int64

    @contextlib.contextmanager
    def _hook(output_dir, device_ids):
        import jax

        jax.devices()
        if device_ids:
            ids = (ctypes.c_int64 * len(device_ids))(*device_ids)
            rc = lib.axon_start_nrt_profile(ids, len(device_ids))
        else:
            rc = lib.axon_start_nrt_profile(None, 0)
        if rc != 0:
            raise RuntimeError(f"axon_start_nrt_profile rc={rc}")
        try:
            yield
        finally:
            n = lib.axon_stop_nrt_profile(str(output_dir).encode())
            print(f"profile: {n} file(s) written to {output_dir}")

    mod = types.ModuleType("antenv.axon_hooks")
    mod.set_axon_ntff_profile_hook = lambda h: None
    mod.get_axon_ntff_profile_hook = lambda: _hook
    sys.modules["antenv.axon_hooks"] = mod


class Cfg:
    """Per-core geometry. Full size: E=1024, SQ=2048, SK=4096."""

    def __init__(self, E=EMB, SQ=S_FULL // 2, SK=S_FULL, n_cores=N_CORES,
                 n_stg=8, qc_size=512, lag=3):
        self.E = E
        self.SQ = SQ             # per-core query rows
        self.SK = SK             # kv rows (full batch)
        self.SKH = SK // 2       # per half
        self.n_cores = n_cores
        self.EC = E // 128       # e-chunks
        self.NBH = self.SKH // 128   # k-blocks per half
        self.NKB = 2 * self.NBH      # k-blocks total
        self.QC = min(qc_size, SQ)
        self.NQC = SQ // self.QC
        self.NJQ = self.QC // 128    # q 128-blocks per q-chunk
        self.n_stg = n_stg       # kv projection chunking (per half)
        assert self.NBH % n_stg == 0
        self.BPS = self.NBH // n_stg      # k-blocks per (stage, half)
        self.KC = self.BPS * 128          # kv rows per (stage, half)
        self.NP = (self.NQC + 1) // 2     # q-chunk pairs
        self.lag = lag
        # PSUM slot layout for P0's 12 AV accumulators: 2 banks x 7 slots
        self.SLOTS_PER_BANK = 7


def build_nc(cfg: Cfg) -> bacc.Bacc:
    E, SQ = cfg.E, cfg.SQ
    EC, NS, BPS, KC = cfg.EC, cfg.n_stg, cfg.BPS, cfg.KC
    QC, NQC, NP, NJQ = cfg.QC, cfg.NQC, cfg.NP, cfg.NJQ
    SPB = cfg.SLOTS_PER_BANK
    # Wq/bq are pre-scaled by SA = 128*log2e/sqrt(DK) on the host, so the
    # score PSUM holds stt = score * SA and:
    #   ScalarE: exp(score/sqrt(DK)) = exp(stt * ASCL),  ASCL = 1/(128*log2e)
    #   VectorE: bf16 bits = round(stt + SB)  (Schraudolph 2^x, single add)
    ASCL = 1.0 / (128.0 * LOG2E)
    SB = 128.0 * (127.0 - 0.043)

    nc = bacc.Bacc("TRN2", target_bir_lowering=False, debug=False,
                   num_devices=cfg.n_cores)

    # x1l: [s][p][c][h][z]  (per partition: one contiguous run per stage)
    x1l = nc.declare_dram_parameter("x1l", [NS * 128 * EC * 2 * KC], BF16,
                                    isOutput=False)
    # x2l: [pr][hh][p][c][z]
    x2l = nc.declare_dram_parameter("x2l", [NP * 128 * EC * 2 * QC], BF16,
                                    isOutput=False)
    wkv = nc.declare_dram_parameter("wkv", [128, EC * 128], BF16, isOutput=False)
    wq1 = nc.declare_dram_parameter("wq1", [128, EC * 64], BF16, isOutput=False)
    bq2 = nc.declare_dram_parameter("bq2", [128, 1], F32, isOutput=False)
    bkv = nc.declare_dram_parameter("bkv", [128, 1], F32, isOutput=False)
    bvk = nc.declare_dram_parameter("bvk", [128, 1], F32, isOutput=False)
    idbf = nc.declare_dram_parameter("idbf", [128, 128], BF16, isOutput=False)
    # unnormalized AV in [q, d] orientation + denominator col 64.
    # cols 0:1024  = P0 (qc 0..2): slot qc*4+jq at col (slot//7)*512 +
    #                (slot%7)*65 (bank-aligned stride so no matmul output
    #                straddles a PSUM bank)
    # cols 1024:1284 = P1 (qc 3): jq*65
    outt = nc.declare_dram_parameter("outt", [128, 2 * 512 + NJQ * 65],
                                     F32, isOutput=True)

    x1v = x1l.ap().rearrange("(s p c h z) -> s p c h z",
                             s=NS, p=128, c=EC, h=2)
    x2v = x2l.ap().rearrange("(r h p c z) -> r h p c z",
                             r=NP, h=2, p=128, c=EC)

    with tile.TileContext(nc) as tc:
        with contextlib.ExitStack() as ctx:
            # ---------------- pools ----------------
            const_pool = ctx.enter_context(tc.tile_pool(name="const", bufs=1))
            x1s_pool = ctx.enter_context(tc.tile_pool(name="x1s", bufs=6))
            x2_pool = ctx.enter_context(tc.tile_pool(name="x2", bufs=4))
            kv_pool = ctx.enter_context(tc.tile_pool(name="kv", bufs=1))
            pt_pool = ctx.enter_context(tc.tile_pool(name="pt", bufs=6))
            acc_pool = ctx.enter_context(tc.tile_pool(name="acc", bufs=2))
            # PSUM: st pool (6 banks) also hosts warm/pkv/pv/pq; av pool
            # (2 banks) holds the phase accumulators. 8 banks total.
            st_pool = ctx.enter_context(
                tc.tile_pool(name="st", bufs=3, space="PSUM"))
            av_pool = ctx.enter_context(
                tc.tile_pool(name="av", bufs=1, space="PSUM"))

            # ---------------- PE warm-up (no DMA dependency) --------------
            # HAM un-throttles after ~3.4us of sustained PE activity; burn
            # matmuls on a memset tile right after the framework preamble so
            # the first real matmuls run at 2.4 GHz.
            dummy = const_pool.tile([128, 128], BF16, tag="dummy")
            nc.vector.memset(dummy[:], 0.0)
            warm = st_pool.tile([128, 128], F32, tag="st", name="warm")
            for _ in range(34):
                nc.tensor.matmul(warm[:], dummy[:], dummy[:],
                                 start=True, stop=True)

            # ---------------- DMA issue (prologue) ----------------
            # Everything data goes on the sync ring (HWDGE) in strict need
            # order (per-ring FIFO delivery): x1 stage 0 (finest split
            # first), x2 chunks 0-2, x1 stages 1..7, x2 chunk 3. Outputs are
            # issued on sync too (the queue is empty by then). Consts go on
            # scalar (small, flat APs -> fat descriptors).
            HC = EC // 2
            wkv_sb = const_pool.tile([128, EC * 128], BF16, tag="wkv")
            nc.scalar.dma_start(wkv_sb[:], wkv.ap())
            id_bf = const_pool.tile([128, 128], BF16, tag="id_bf")
            nc.scalar.dma_start(id_bf[:], idbf.ap())
            bkv_sb = const_pool.tile([128, 1], F32, tag="bkv")
            nc.scalar.dma_start(bkv_sb[:], bkv.ap())
            bvk_sb = const_pool.tile([128, 1], F32, tag="bvk")
            nc.scalar.dma_start(bvk_sb[:], bvk.ap())
            wq1_sb = const_pool.tile([128, EC * 64], BF16, tag="wq1")
            nc.scalar.dma_start(wq1_sb[:], wq1.ap())
            bq2_sb = const_pool.tile([128, 1], F32, tag="bq2")
            nc.scalar.dma_start(bq2_sb[:], bq2.ap())
            # derived stationaries (DVE, ~1us total, off critical path)
            wkv_v = wkv_sb[:].rearrange("p (c h d) -> p c h d", h=2, d=64)
            wvk_sb = const_pool.tile([128, EC, 2, 64], BF16, tag="wvk")
            nc.vector.tensor_copy(wvk_sb[:, :, 0, :], wkv_v[:, :, 1, :])
            nc.vector.tensor_copy(wvk_sb[:, :, 1, :], wkv_v[:, :, 0, :])
            wq1_v = wq1_sb[:].rearrange("p (c d) -> p c d", d=64)
            wq2_sb = const_pool.tile([128, EC, 2, 64], BF16, tag="wq2")
            nc.vector.tensor_copy(wq2_sb[:, :, 0, :], wq1_v[:])
            nc.vector.tensor_copy(wq2_sb[:, :, 1, :], wq1_v[:])

            x1h = {}       # (s, half) -> tile [128, HC, 2, KC]
            x1q = {}       # finest-split tiles for stage 0 half 0
            x2h = {}       # chunk j (= qc) -> tile [128, EC, QC]

            def load_x2(j):
                t = x2_pool.tile([128, EC, QC], BF16, tag="x2", name=f"x2c{j}")
                nc.sync.dma_start(t[:], x2v[j // 2, j % 2])
                x2h[j] = t

            for q in (0, 1):
                t = x1s_pool.tile([128, 1, 2, KC], BF16, tag="x1q",
                                  name=f"x1q{q}", bufs=2)
                nc.sync.dma_start(t[:], x1v[0, :, q:q + 1])
                x1q[q] = t
            t = x1s_pool.tile([128, HC - 2, 2, KC], BF16, tag="x1q2",
                              name="x1q2", bufs=1)
            nc.sync.dma_start(t[:], x1v[0, :, 2:HC])
            x1q[2] = t
            t = x1s_pool.tile([128, HC, 2, KC], BF16, tag="x1s", name="x1s0h1",
                              bufs=15)
            nc.sync.dma_start(t[:], x1v[0, :, HC:EC])
            x1h[(0, 1)] = t
            load_x2(0)
            load_x2(1)
            for s in range(1, NS):
                for hf in (0, 1):
                    t = x1s_pool.tile([128, HC, 2, KC], BF16, tag="x1s",
                                      name=f"x1s{s}h{hf}", bufs=15)
                    nc.sync.dma_start(t[:], x1v[s, :, hf * HC:(hf + 1) * HC])
                    x1h[(s, hf)] = t
                if s == 1:
                    load_x2(2)
            load_x2(3)

            def x1slab(s, c):
                if s == 0 and c < HC:
                    if c < 2:
                        return x1q[c][:, 0]
                    return x1q[2][:, c - 2]
                return x1h[(s, c // HC)][:, c % HC]

            # ---------------- persistent tiles ----------------
            # ckv[s][h]: [128, KC] combined K^T/V^T rows (bias applied).
            #   h=0: K rows 0:64, V rows 64:128;  h=1: V rows 0:64, K 64:128.
            ckv = [[kv_pool.tile([128, KC], BF16, tag=f"ckv{s}{h}",
                                 name=f"ckv{s}{h}") for h in (0, 1)]
                   for s in range(NS)]
            # v_stage[s]: [128, 2*BPS*65] V|ones blocks (h0 blocks then h1)
            v_stage = [kv_pool.tile([128, 2 * BPS * 65], BF16, tag=f"vs{s}",
                                    name=f"vs{s}") for s in range(NS)]
            qt2 = [kv_pool.tile([128, QC], BF16, tag=f"qt{q}", name=f"qt{q}")
                   for q in range(NQC)]

            # ---------------- phase 1 pieces ----------------
            wkv_cv = wkv_sb[:].rearrange("p (c x) -> p c x", x=128)
            wvk_cv = wvk_sb[:].rearrange("p c h d -> p c (h d)")
            wq2_cv = wq2_sb[:].rearrange("p c h d -> p c (h d)")

            def kv_proj_h(s, h):
                # one half-projection as a complete alloc->MMs->evac block so
                # the st ring never holds more than one kv tile at a time;
                # evac/copy work is split across ScalarE (h0) / VectorE (h1)
                ws = {0: wkv_cv, 1: wvk_cv}
                pkv = st_pool.tile([128, KC], F32, tag="st",
                                   name=f"pkv{s}_{h}")
                for c in range(EC):
                    nc.tensor.matmul(pkv[:], ws[h][:, c],
                                     x1slab(s, c)[:, h, :],
                                     start=(c == 0), stop=(c == EC - 1))
                if h == 0:
                    nc.scalar.activation(ckv[s][h][:], pkv[:],
                                         AF.Identity, bias=bkv_sb[:],
                                         scale=1.0)
                else:
                    nc.vector.tensor_scalar(ckv[s][h][:], pkv[:],
                                            bvk_sb[:], None, ALU.add)

            def kv_vstage_h(s, h):
                vrows = slice(64, 128) if h == 0 else slice(0, 64)
                ident = id_bf[64:128, 64:128] if h == 0 else id_bf[0:64, 0:64]
                pv = st_pool.tile([128, BPS * 64], BF16, tag="st",
                                  name=f"pv{s}_{h}")
                for j in range(BPS):
                    nc.tensor.transpose(pv[:, j * 64:(j + 1) * 64],
                                        ckv[s][h][vrows, j * 128:(j + 1) * 128],
                                        ident)
                vslab = v_stage[s][:, h * BPS * 65:(h + 1) * BPS * 65].rearrange(
                    "p (j d) -> p j d", d=65)
                if h == 0:
                    nc.scalar.copy(vslab[:, :, 0:64],
                                   pv[:].rearrange("p (j d) -> p j d", d=64))
                else:
                    nc.vector.tensor_copy(
                        vslab[:, :, 0:64],
                        pv[:].rearrange("p (j d) -> p j d", d=64))
                nc.vector.memset(vslab[:, :, 64:65], 1.0)

            def q_chunk(j):
                pq = st_pool.tile([128, QC], F32, tag="st", name=f"pq{j}")
                for c in range(EC):
                    nc.tensor.matmul(pq[:], wq2_cv[:, c], x2h[j][:, c, :],
                                     start=(c == 0), stop=(c == EC - 1))
                nc.vector.tensor_scalar(qt2[j][:], pq[:], bq2_sb[:], None,
                                        ALU.add)

            # ---------------- phase 2: attention ----------------
            # stage-outer unit order; P0 = qc 0..2 (one 2-bank PSUM tile of
            # 12 slot-accumulators), P1 = qc 3 (1 bank). qc2's stage-s units
            # are delayed by one stage so they never head-of-line block on
            # the later x2 chunk-2 DMA; the next stage's kv half-blocks are
            # woven between units so the score ring keeps depth >= 2.
            actions = []
            for s in range(NS):
                block = [("u", (s, 0, 0)), ("u", (s, 0, 1)),
                         ("u", (s, 1, 0)), ("u", (s, 1, 1))]
                if s >= 1:
                    block += [("u", (s - 1, 2, 0)), ("u", (s - 1, 2, 1))]
                nxt = []
                if s + 1 < NS:
                    nxt = [("kvh", s + 1, 0), ("kvh", s + 1, 1),
                           ("vsh", s + 1, 0), ("vsh", s + 1, 1)]
                if s == 0:
                    nxt = [("qc", 1)] + nxt
                if s == 1:
                    nxt = [("qc", 2)] + nxt
                if s == NS - 2:
                    nxt = nxt + [("qc", 3)]
                woven = []
                for i, u in enumerate(block):
                    woven.append(u)
                    if i < len(nxt):
                        woven.append(nxt[i])
                woven += nxt[len(block):]
                actions += woven
            actions += [("u", (NS - 1, 2, 0)), ("u", (NS - 1, 2, 1))]
            actions += [("u", (s, 3, pos)) for s in range(NS)
                        for pos in range(BPS)]
            units = [a[1] for a in actions if a[0] == "u"]
            NU = len(units)

            def av_slot(qc, jq):
                if qc == 3:
                    return (1, None, jq)       # phase 1, [128, NJQ, 65]
                sl = qc * NJQ + jq
                return (0, sl // SPB, sl % SPB)

            writes = []
            for ui, (s, qc, pos) in enumerate(units):
                for h in (0, 1):
                    for jq in range(NJQ):
                        ph, b, idx = av_slot(qc, jq)
                        writes.append((ui, h, jq, (ph, b)))
            first_w = {}
            last_w = {}
            for w in writes:
                key = w[3]
                if key not in first_w:
                    first_w[key] = w[:3]
                last_w[key] = w[:3]

            DVE_MOD = {1, 3, 5, 7, 9, 11, 13}  # 7 of 16 units -> VectorE exp

            state = {}
            av_t = {}   # phase -> tile

            def emit_scores(ui):
                s, qc, pos = units[ui]
                stt = st_pool.tile([128, 2, QC], F32, tag="st",
                                   name=f"st{ui}")
                for h in (0, 1):
                    nc.tensor.matmul(
                        stt[:, h, :],
                        ckv[s][h][h * 64:(h + 1) * 64,
                                  pos * 128:(pos + 1) * 128],
                        qt2[qc][h * 64:(h + 1) * 64, :],
                        start=True, stop=True)
                pt = pt_pool.tile([128, 2, QC], BF16, tag="pt",
                                  name=f"pt{ui}")
                stf = stt[:].rearrange("p h q -> p (h q)")
                ptf = pt[:].rearrange("p h q -> p (h q)")
                if ui % 16 in DVE_MOD:
                    nc.vector.tensor_scalar(ptf[:].bitcast(I16), stf[:],
                                            float(SB), None, ALU.add)
                else:
                    nc.scalar.activation(ptf[:], stf[:], AF.Exp,
                                         scale=float(ASCL))
                state[ui] = pt

            def emit_av(ui):
                s, qc, pos = units[ui]
                pt = state.pop(ui)
                ph = 1 if qc == 3 else 0
                if ph not in av_t:
                    if ph == 0:
                        av_t[0] = av_pool.tile([128, 2, 512], F32,
                                               tag="av", name="avP0")
                    else:
                        av_t[1] = av_pool.tile([128, NJQ, 65], F32,
                                               tag="av", name="avP1")
                av = av_t[ph]
                for h in (0, 1):
                    vcol = (h * BPS + pos) * 65
                    vblk = v_stage[s][:, vcol:vcol + 65]
                    for jq in range(NJQ):
                        p_, b, idx = av_slot(qc, jq)
                        out = (av[:, idx, :] if p_ == 1
                               else av[:, b, idx * 65:idx * 65 + 65])
                        key = (p_, b)
                        nc.tensor.matmul(
                            out, pt[:, h, jq * 128:(jq + 1) * 128], vblk,
                            start=(first_w[key] == (ui, h, jq)),
                            stop=(last_w[key] == (ui, h, jq)),
                            skip_group_check=True)
                if (s, pos) == (NS - 1, BPS - 1) and qc == (2 if ph == 0 else 3):
                    if ph == 0:
                        acc = acc_pool.tile([128, 2 * 512], F32,
                                            tag="acc", name="accP0")
                        nc.scalar.copy(acc[:],
                                       av[:].rearrange("p b d -> p (b d)"))
                        nc.sync.dma_start(outt.ap()[:, 0:2 * 512], acc[:])
                    else:
                        acc = acc_pool.tile([128, NJQ * 65], F32,
                                            tag="acc", name="accP1")
                        nc.scalar.copy(acc[:],
                                       av[:].rearrange("p j d -> p (j d)"))
                        nc.sync.dma_start(
                            outt.ap()[:, 2 * 512:2 * 512 + NJQ * 65],
                            acc[:])

            # emission: prologue (stage 0 + qc0), then the woven action plan
            LAG = cfg.lag
            kv_proj_h(0, 0)
            kv_proj_h(0, 1)
            q_chunk(0)
            kv_vstage_h(0, 0)
            kv_vstage_h(0, 1)
            ucount = 0
            for act in actions:
                if act[0] == "u":
                    emit_scores(ucount)
                    ucount += 1
                    if ucount - 1 >= LAG:
                        emit_av(ucount - 1 - LAG)
                    if ucount <= 40:
                        # HAM warm-keeper: during the DMA-paced era the PE
                        # micro-idles enough to oscillate the clock gate
                        # (cold matmuls run at half rate). Dummy LDWEIGHTS
                        # streams keep the array active; they touch no PSUM
                        # and every real matmul loads its own weights.
                        for _ in range(6):
                            nc.tensor.ldweights(dummy[:])
                elif act[0] == "kvh":
                    kv_proj_h(act[1], act[2])
                elif act[0] == "vsh":
                    kv_vstage_h(act[1], act[2])
                elif act[0] == "qc":
                    q_chunk(act[1])
            for ui in range(NU - LAG, NU):
                emit_av(ui)

    nc.compile()
    return nc


# ----------------------------------------------------------------------------
# host side

def _to_bf16(a):
    import ml_dtypes
    return np.asarray(a).astype(ml_dtypes.bfloat16)


def prep_consts(cfg: Cfg, Wq, bq, Wk, bk, Wv, bv):
    EC = cfg.EC
    # fold SA = 128*log2e/sqrt(DK) into the Q projection so the device's
    # Schraudolph exp is a single add (see build_nc)
    SA = 128.0 * LOG2E / np.sqrt(DK)
    Wq = np.asarray(Wq) * SA
    bq = np.asarray(bq) * SA
    wq_r = _to_bf16(Wq).reshape(EC, 128, DK).transpose(1, 0, 2)  # [128, EC, 64]
    wk_r = _to_bf16(Wk).reshape(EC, 128, DK).transpose(1, 0, 2)
    wv_r = _to_bf16(Wv).reshape(EC, 128, DK).transpose(1, 0, 2)
    wq1 = wq_r.reshape(128, EC * 64)
    wkv = np.concatenate([wk_r, wv_r], axis=2).reshape(128, EC * 128)
    bq2 = np.concatenate([bq, bq]).reshape(128, 1).astype(np.float32)
    bkv = np.concatenate([bk, bv]).reshape(128, 1).astype(np.float32)
    bvk = np.concatenate([bv, bk]).reshape(128, 1).astype(np.float32)
    idbf = _to_bf16(np.eye(128, dtype=np.float32))
    return {
        "wq1": np.ascontiguousarray(wq1), "wkv": np.ascontiguousarray(wkv),
        "bq2": bq2, "bkv": bkv, "bvk": bvk,
        "idbf": np.ascontiguousarray(idbf),
    }


def shard_inputs(cfg: Cfg, input1, input2, Wq, bq, Wk, bk, Wv, bv):
    consts = prep_consts(cfg, Wq, bq, Wk, bk, Wv, bv)
    i1 = _to_bf16(input1)
    i2 = _to_bf16(input2)
    in_maps = []
    for c in range(cfg.n_cores):
        b = c // 2
        r = c % 2
        # x1: [E, SK] -> [s][p][ch][h][z]   (k = h*SKH + s*KC + z)
        x1tc = i1[b].T.reshape(cfg.EC, 128, 2, cfg.n_stg, cfg.KC)
        x1lv = np.ascontiguousarray(
            x1tc.transpose(3, 1, 0, 2, 4)).reshape(-1)
        # x2: [E, SQ] -> [pr][hh][p][ch][z]  (q = pr*2*QC + hh*QC + z)
        x2tc = i2[b, r * cfg.SQ:(r + 1) * cfg.SQ, :].T
        a = x2tc.reshape(cfg.EC, 128, cfg.NP, 2, cfg.QC)
        x2lv = np.ascontiguousarray(a.transpose(2, 3, 1, 0, 4)).reshape(-1)
        m = {"x1l": x1lv, "x2l": x2lv}
        m.update(consts)
        in_maps.append(m)
    return in_maps


_NC_CACHE = {}


def get_nc(cfg: Cfg) -> bacc.Bacc:
    key = (cfg.E, cfg.SQ, cfg.SK, cfg.n_cores, cfg.n_stg, cfg.QC, cfg.lag)
    if key not in _NC_CACHE:
        _NC_CACHE[key] = build_nc(cfg)
    return _NC_CACHE[key]


def run(inputs: dict, trace: bool = False):
    """Run on hardware; returns (full_output [B,S,DK] f32, exec_time_ns)."""
    cfg = Cfg()
    nc = get_nc(cfg)
    in_maps = shard_inputs(cfg, **inputs)
    if trace:
        install_ntff_hook()
    res = run_bass_kernel_spmd(nc, in_maps, list(range(cfg.n_cores)),
                               trace=trace)
    SPB = cfg.SLOTS_PER_BANK
    full = np.empty((B_FULL, S_FULL, DK), dtype=np.float32)
    for c in range(cfg.n_cores):
        b = c // 2
        r = c % 2
        ot = np.asarray(res.results[c]["outt"])   # [128, 1024 + NJQ*65]
        o = np.empty((cfg.NQC, cfg.NJQ, 128, DK), dtype=np.float32)
        for qc in range(cfg.NQC):
            for jq in range(cfg.NJQ):
                if qc == 3:
                    col = 2 * 512 + jq * 65
                else:
                    slot = qc * cfg.NJQ + jq
                    col = (slot // cfg.SLOTS_PER_BANK) * 512 + \
                          (slot % cfg.SLOTS_PER_BANK) * 65
                blk = ot[:, col:col + 65]
                o[qc, jq] = blk[:, 0:64] / blk[:, 64:65]
        full[b, r * cfg.SQ:(r + 1) * cfg.SQ, :] = o.transpose(
            0, 2, 1, 3)[:, :, :, :].reshape(-1, DK) if False else \
            o.reshape(cfg.NQC, cfg.NJQ, 128, DK).transpose(
                0, 1, 2, 3).reshape(cfg.SQ, DK)
    return full, res.exec_time_ns


def kernel(**inputs) -> np.ndarray:
    inputs = {k: np.asarray(v, dtype=np.float32) for k, v in inputs.items()}
    full, _ = run(inputs, trace=False)
    return full


if __name__ == "__main__":
    rng = np.random.default_rng(0)
    inputs = {
        "input1": rng.standard_normal((B_FULL, S_FULL, EMB), dtype=np.float32),
        "input2": rng.standard_normal((B_FULL, S_FULL, EMB), dtype=np.float32),
        "Wq": rng.uniform(-1 / 32, 1 / 32, (EMB, DK)).astype(np.float32),
        "bq": rng.uniform(-1 / 32, 1 / 32, (DK,)).astype(np.float32),
        "Wk": rng.uniform(-1 / 32, 1 / 32, (EMB, DK)).astype(np.float32),
        "bk": rng.uniform(-1 / 32, 1 / 32, (DK,)).astype(np.float32),
        "Wv": rng.uniform(-1 / 32, 1 / 32, (EMB, DK)).astype(np.float32),
        "bv": rng.uniform(-1 / 32, 1 / 32, (DK,)).astype(np.float32),
    }
    out = kernel(**inputs)
    print("out", out.shape, out.dtype)
